# revision 1
# baseline (speedup 1.0000x reference)
"""Trainium2 Bass kernel for nn_DenseAttention (sparse_attention, C=31, B=D=1024).

Strategy (class-parallel over 8 NeuronCores):
- Each core handles 4 classes (core 7: 3 real + 1 zero dummy).
- Per class on device: xBT = K_c^T-weighted matmul of x (fp16 hi/lo split,
  3 matmul terms, fp32 PSUM accumulate), then xBBx logits on the allowed
  cross-domain half, label-equality masking, E = exp(logits - 200), row sums
  and boundary-split partial row sums, AE = sum_c E_c.
- The reference's softmax is a raw reshape [B,B,C] -> [C, B*B]: softmax groups
  are 31 chunks of 2^20 flat elements crossing class boundaries. Group
  membership of (p=i*B+j, c) is (31p+c)>>20; per class each group is a
  contiguous p-range, so group sums are assembled on the host from whole-row
  sums plus lo-part partial sums at the <=30 boundary rows per class.
- exp shift is the constant 200 (any per-group-constant shift cancels in the
  softmax ratio; 200 keeps everything in fp32 range and reproduces the
  reference's masked-element underflow-to-zero behaviour exactly).
- Host: sums s_g in fp64, out = (sum_cores AE) / s_{g0(p)} plus corrections at
  the <=30 flat positions per group whose true group differs from g0(p).
"""

import functools

import numpy as np

import concourse.mybir as mybir
import concourse.tile as tile
from concourse import bacc
from concourse.bass_utils import run_bass_kernel_spmd

C, B, D = 31, 1024, 1024
NCORES = 8
CPAD = 4
MHAT = 200.0
SC = 2048.0
ISC = float(1.0 / SC)
M_FLAT = 1 << 20
F32 = mybir.dt.float32
F16 = mybir.dt.float16
EXP = mybir.ActivationFunctionType.Exp
ALU = mybir.AluOpType


def _pc(c, g):
    """First p with (31p + c) >= g * 2^20."""
    return (g * M_FLAT - c + 30) // 31


@functools.lru_cache(maxsize=1)
def _build():
    nc = bacc.Bacc("TRN2", target_bir_lowering=False, debug=False,
                   num_devices=NCORES)
    xth_d = nc.dram_tensor("xth", [8, 128, 1024], F16, kind="ExternalInput")
    xtl_d = nc.dram_tensor("xtl", [8, 128, 1024], F16, kind="ExternalInput")
    khi_d = nc.dram_tensor("khi", [CPAD, 8, 128, 1024], F16, kind="ExternalInput")
    klo_d = nc.dram_tensor("klo", [CPAD, 8, 128, 1024], F16, kind="ExternalInput")
    labi_d = nc.dram_tensor("labi", [128, CPAD * 8], F32, kind="ExternalInput")
    labj_d = nc.dram_tensor("labj", [128, CPAD * 512], F32, kind="ExternalInput")
    tvec_d = nc.dram_tensor("tvec", [128, CPAD * 8], F32, kind="ExternalInput")
    bias_d = nc.dram_tensor("biasc", [128, CPAD * 8], F32, kind="ExternalInput")
    iota_d = nc.dram_tensor("iota", [128, 512], F32, kind="ExternalInput")

    # upper cross block only (E is symmetric; host mirrors the lower block)
    ae_d = nc.dram_tensor("out_ae", [128, 4 * 512], F32, kind="ExternalOutput")
    rse_d = nc.dram_tensor("out_rse", [128, CPAD * 8], F32, kind="ExternalOutput")
    rslo_d = nc.dram_tensor("out_rslo", [128, CPAD * 8], F32, kind="ExternalOutput")
    oute_d = nc.dram_tensor("out_e", [128, CPAD * 4 * 512], F32,
                            kind="ExternalOutput")

    with tile.TileContext(nc) as tc:
        with (
            tc.tile_pool(name="persist", bufs=1) as pp,
            tc.tile_pool(name="kpool", bufs=2) as kp,
            tc.tile_pool(name="work", bufs=3) as wp,
            tc.tile_pool(name="psum", bufs=3, space="PSUM") as ps,
        ):
            xth_t = pp.tile([128, 8 * 1024], F16)
            xtl_t = pp.tile([128, 8 * 1024], F16)
            xbh_t = pp.tile([128, 8 * 1024], F16)
            xbl_t = pp.tile([128, 8 * 1024], F16)
            ae_t = pp.tile([128, 4 * 512], F32)
            labi_t = pp.tile([128, CPAD * 8], F32)
            labj_t = pp.tile([128, CPAD * 512], F32)
            tvec_t = pp.tile([128, CPAD * 8], F32)
            bias_t = pp.tile([128, CPAD * 8], F32)
            iota_t = pp.tile([128, 512], F32)
            rse_t = pp.tile([128, CPAD * 8], F32)
            rslo_t = pp.tile([128, CPAD * 8], F32)
            b200_t = pp.tile([128, 1], F32)

            for dc in range(8):
                nc.sync.dma_start(out=xth_t[:, dc * 1024:(dc + 1) * 1024],
                                  in_=xth_d[dc])
                nc.sync.dma_start(out=xtl_t[:, dc * 1024:(dc + 1) * 1024],
                                  in_=xtl_d[dc])
            nc.sync.dma_start(out=bias_t[:], in_=bias_d[:])
            nc.vector.memset(b200_t[:], -MHAT)
            nc.vector.memset(ae_t[:], 0.0)
            nc.vector.memset(rse_t[:], 0.0)
            nc.vector.memset(rslo_t[:], 0.0)


            for cl in range(CPAD):
                # ---- matmul1: xBT[e, i] = sum_d K[d,e] * xT[d,i] (+bias) ----
                for et in range(8):
                    kh_t = kp.tile([128, 8 * 128], F16, tag="kh")
                    kl_t = kp.tile([128, 8 * 128], F16, tag="kl")
                    nc.gpsimd.dma_start(out=kh_t[:], in_=khi_d[cl, et])
                    nc.gpsimd.dma_start(out=kl_t[:], in_=klo_d[cl, et])
                    p1a = ps.tile([128, 512], F32, tag="p1")
                    p1b = ps.tile([128, 512], F32, tag="p1")
                    p2a = ps.tile([128, 512], F32, tag="p2")
                    p2b = ps.tile([128, 512], F32, tag="p2")
                    p1s = [p1a, p1b]
                    p2s = [p2a, p2b]
                    # all consumers of each weight tile back-to-back
                    for dc in range(8):
                        w = kh_t[:, dc * 128:(dc + 1) * 128]
                        for ih in range(2):
                            nc.tensor.matmul(
                                out=p1s[ih][:], lhsT=w,
                                rhs=xth_t[:, dc * 1024 + ih * 512:
                                          dc * 1024 + ih * 512 + 512],
                                start=(dc == 0), stop=(dc == 7))
                        for ih in range(2):
                            nc.tensor.matmul(
                                out=p2s[ih][:], lhsT=w,
                                rhs=xtl_t[:, dc * 1024 + ih * 512:
                                          dc * 1024 + ih * 512 + 512],
                                start=(dc == 0), stop=False)
                    for dc in range(8):
                        w = kl_t[:, dc * 128:(dc + 1) * 128]
                        for ih in range(2):
                            nc.tensor.matmul(
                                out=p2s[ih][:], lhsT=w,
                                rhs=xth_t[:, dc * 1024 + ih * 512:
                                          dc * 1024 + ih * 512 + 512],
                                start=False, stop=(dc == 7))
                    for ih in range(2):
                        p1 = p1s[ih]
                        p2 = p2s[ih]
                        vtmp = wp.tile([128, 512], F32, tag="vtmp")
                        vfull = wp.tile([128, 512], F32, tag="vfull")
                        dtmp = wp.tile([128, 512], F32, tag="dtmp")
                        nc.vector.tensor_scalar(
                            out=vtmp[:], in0=p2[:], scalar1=ISC, scalar2=None,
                            op0=ALU.mult)
                        nc.vector.scalar_tensor_tensor(
                            out=vfull[:], in0=p1[:],
                            scalar=bias_t[:, cl * 8 + et:cl * 8 + et + 1],
                            in1=vtmp[:], op0=ALU.add, op1=ALU.add)
                        osl = slice(et * 1024 + ih * 512, et * 1024 + ih * 512 + 512)
                        nc.scalar.copy(out=xbh_t[:, osl], in_=vfull[:])
                        nc.vector.scalar_tensor_tensor(
                            out=dtmp[:], in0=xbh_t[:, osl], scalar=-1.0,
                            in1=vfull[:], op0=ALU.mult, op1=ALU.add)
                        nc.vector.tensor_scalar(
                            out=xbl_t[:, osl], in0=dtmp[:], scalar1=SC,
                            scalar2=None, op0=ALU.mult)

                if cl == 0:
                    # M2-only inputs: issued here so the preamble DMA queue
                    # holds only what the first matmuls need
                    nc.sync.dma_start(out=labi_t[:], in_=labi_d[:])
                    nc.sync.dma_start(out=labj_t[:], in_=labj_d[:])
                    nc.sync.dma_start(out=tvec_t[:], in_=tvec_d[:])
                    nc.sync.dma_start(out=iota_t[:], in_=iota_d[:])

                # ---- matmul2 + mask + exp + sums, upper cross block only ----
                for it in range(4):
                    jlo = 512
                    q1 = ps.tile([128, 512], F32, tag="p1")
                    q2 = ps.tile([128, 512], F32, tag="p2")
                    for ec in range(8):
                        ioff = ec * 1024 + it * 128
                        joff = ec * 1024 + jlo
                        w = xbh_t[:, ioff:ioff + 128]
                        nc.tensor.matmul(
                            out=q1[:], lhsT=w,
                            rhs=xbh_t[:, joff:joff + 512],
                            start=(ec == 0), stop=(ec == 7))
                        nc.tensor.matmul(
                            out=q2[:], lhsT=w,
                            rhs=xbl_t[:, joff:joff + 512],
                            start=(ec == 0), stop=False)
                    for ec in range(8):
                        ioff = ec * 1024 + it * 128
                        joff = ec * 1024 + jlo
                        nc.tensor.matmul(
                            out=q2[:], lhsT=xbl_t[:, ioff:ioff + 128],
                            rhs=xbh_t[:, joff:joff + 512],
                            start=False, stop=(ec == 7))
                    vtmp = wp.tile([128, 512], F32, tag="vtmp")
                    vfull = wp.tile([128, 512], F32, tag="vfull")
                    eqt = wp.tile([128, 512], F32, tag="eqt")
                    mt = wp.tile([128, 512], F32, tag="mt")
                    ext = wp.tile([128, 512], F32, tag="ext")
                    scr = wp.tile([128, 512], F32, tag="scr")
                    nc.vector.tensor_scalar(
                        out=vtmp[:], in0=q2[:], scalar1=ISC, scalar2=None,
                        op0=ALU.mult)
                    nc.vector.tensor_tensor(
                        out=vfull[:], in0=q1[:], in1=vtmp[:], op=ALU.add)
                    nc.vector.tensor_scalar(
                        out=eqt[:], in0=labj_t[:, cl * 512:cl * 512 + 512],
                        scalar1=labi_t[:, cl * 8 + it:cl * 8 + it + 1],
                        scalar2=None, op0=ALU.is_equal)
                    nc.vector.tensor_tensor(
                        out=mt[:], in0=vfull[:], in1=eqt[:], op=ALU.mult)
                    nc.scalar.activation(
                        out=ext[:], in_=mt[:], func=EXP, bias=b200_t[:],
                        scale=1.0, accum_out=rse_t[:, cl * 8 + it:cl * 8 + it + 1])
                    nc.vector.scalar_tensor_tensor(
                        out=scr[:], in0=iota_t[:, 0:512],
                        scalar=tvec_t[:, cl * 8 + it:cl * 8 + it + 1],
                        in1=ext[:], op0=ALU.is_lt, op1=ALU.mult,
                        accum_out=rslo_t[:, cl * 8 + it:cl * 8 + it + 1])
                    asl = slice(it * 512, it * 512 + 512)
                    nc.vector.tensor_tensor(
                        out=ae_t[:, asl], in0=ae_t[:, asl], in1=ext[:],
                        op=ALU.add)
                    if cl == CPAD - 1:
                        nc.sync.dma_start(out=ae_d[:, asl], in_=ae_t[:, asl])
                    # ship E upper block to host (lower-row sums in fp64 there)
                    eoff = (cl * 4 + it) * 512
                    nc.sync.dma_start(out=oute_d[:, eoff:eoff + 512], in_=ext[:])

            nc.sync.dma_start(out=rse_d[:], in_=rse_t[:])
            nc.sync.dma_start(out=rslo_d[:], in_=rslo_t[:])

    nc.compile()
    return nc


def _split_f16(v):
    hi = v.astype(np.float16)
    lo = ((v.astype(np.float64) - hi.astype(np.float64)) * SC).astype(np.float16)
    return hi, lo


def _core_classes():
    return [list(range(c * 4, min(c * 4 + 4, C))) for c in range(NCORES)]


def _thresholds(c):
    """Per-row j-split T[i] for global class c (0 = no boundary in row)."""
    T = np.zeros(B, np.int64)
    for g in range(1, C):
        p = _pc(c, g)
        i0, t = divmod(p, B)
        if t != 0:
            T[i0] = t
    return T


def _prep_inputs(x, labels, kernel, bias):
    xT = np.ascontiguousarray(x.T)
    xth, xtl = _split_f16(xT)
    xth = np.ascontiguousarray(xth.reshape(8, 128, 1024))
    xtl = np.ascontiguousarray(xtl.reshape(8, 128, 1024))
    iota = np.broadcast_to(np.arange(512, 1024, dtype=np.float32)[None, :],
                           (128, 512)).copy()
    in_maps = []
    for classes in _core_classes():
        k4 = np.zeros((CPAD, D, D), np.float32)
        b4 = np.zeros((CPAD, D), np.float32)
        l4 = np.zeros((B, CPAD), np.int32)
        t4 = np.zeros((CPAD, B), np.int64)
        for cl, c in enumerate(classes):
            k4[cl] = kernel[c]
            b4[cl] = bias[c]
            l4[:, cl] = labels[:, c]
            t4[cl] = _thresholds(c)
        khi, klo = _split_f16(k4)
        # [cl, d, e] -> [cl, et(8), p(128), dc(8), e(128)] laid as [cl,8,128,1024]
        def re(a):
            a = a.reshape(CPAD, 8, 128, 8, 128)          # cl, dc, p, et, e
            a = np.ascontiguousarray(a.transpose(0, 3, 2, 1, 4))  # cl, et, p, dc, e
            return a.reshape(CPAD, 8, 128, 1024)
        labi = l4.reshape(8, 128, CPAD).transpose(1, 2, 0)      # p, cl, it
        labi = np.ascontiguousarray(labi.astype(np.float32)).reshape(128, CPAD * 8)
        labj = np.broadcast_to(
            l4[512:, :].T.astype(np.float32)[None, :, :], (128, CPAD, 512)
        ).reshape(128, CPAD * 512).copy()
        tvec = t4.reshape(CPAD, 8, 128).transpose(2, 0, 1)      # p, cl, it
        tvec = np.ascontiguousarray(tvec.astype(np.float32)).reshape(128, CPAD * 8)
        biasc = b4.reshape(CPAD, 8, 128).transpose(2, 0, 1)     # p, cl, et
        biasc = np.ascontiguousarray(biasc.astype(np.float32)).reshape(128, CPAD * 8)
        in_maps.append(dict(
            xth=xth, xtl=xtl, khi=re(khi), klo=re(klo),
            labi=labi, labj=labj, tvec=tvec, biasc=biasc, iota=iota,
        ))
    return in_maps


def _assemble(results, x, labels, kernel, bias):
    s = np.zeros(C, np.float64)
    AE_tot = np.zeros((B, B), np.float64)
    i_idx = np.arange(B, dtype=np.int64)
    for res, classes in zip(results, _core_classes()):
        # upper cross block [i<512, j>=512]; lower block is its transpose
        up = res["out_ae"].reshape(128, 4, 512).transpose(1, 0, 2).reshape(512, 512)
        AE_tot[:512, 512:] += up
        AE_tot[512:, :512] += up.T
        rse = res["out_rse"].reshape(128, CPAD, 8).transpose(1, 2, 0)\
            .reshape(CPAD, B).astype(np.float64)
        rslo = res["out_rslo"].reshape(128, CPAD, 8).transpose(1, 2, 0)\
            .reshape(CPAD, B).astype(np.float64)
        eb = res["out_e"].reshape(128, CPAD, 4, 512)
        jv = np.arange(512, dtype=np.int64)[:, None]
        for cl, c in enumerate(classes):
            g_row = (31 * (i_idx * B) + c) >> 20
            T = _thresholds(c)
            e_cl = eb[:, cl].transpose(1, 0, 2).reshape(512, 512).astype(np.float64)
            rse[cl][512:] = e_cl.sum(axis=0)
            mlow = (jv < T[512:][None, :]).astype(np.float64)
            rslo[cl][512:] = (e_cl * mlow).sum(axis=0)
            hb = T > 0
            np.add.at(s, g_row[~hb], rse[cl][~hb].astype(np.float64))
            np.add.at(s, g_row[hb], rslo[cl][hb].astype(np.float64))
            np.add.at(s, g_row[hb] + 1,
                      (rse[cl][hb] - rslo[cl][hb]).astype(np.float64))
    p = np.arange(B * B, dtype=np.int64)
    g0 = (31 * p) >> 20
    out = AE_tot * (1.0 / s)[g0].reshape(B, B)

    # corrections at flat positions whose true group g differs from g0(p)
    half = B // 2
    corr = {}  # (i, j) -> list of (c, g)
    for g in range(1, C):
        pB_ = _pc(0, g)
        for c in range(C):
            for pstar in range(_pc(c, g), pB_):
                i, j = divmod(pstar, B)
                cross = (i < half) != (j < half)
                if cross and labels[i, c] == labels[j, c]:
                    corr.setdefault((i, j), []).append((c, g))
    for (i, j), lst in corr.items():
        for c, g in lst:
            vi = x[i].astype(np.float64) @ kernel[c].astype(np.float64) \
                + bias[c].astype(np.float64)
            vj = x[j].astype(np.float64) @ kernel[c].astype(np.float64) \
                + bias[c].astype(np.float64)
            Mij = np.float64(np.float32(vi @ vj))
            E = np.exp(Mij - MHAT)
            out[i, j] += E * (1.0 / s[g] - 1.0 / s[g - 1])
    return out.astype(np.float32)


def _run(inputs, trace=False):
    x = np.asarray(inputs["inputs"], np.float32)
    labels = np.asarray(inputs["labels"])
    kern = np.asarray(inputs["kernel"], np.float32)
    bias = np.asarray(inputs["bias"], np.float32)
    nc = _build()
    in_maps = _prep_inputs(x, labels, kern, bias)
    res = run_bass_kernel_spmd(nc, in_maps, core_ids=list(range(NCORES)),
                               trace=trace)
    out = _assemble(res.results, x, labels, kern, bias)
    return out, res


def kernel(**inputs) -> np.ndarray:
    return _run(inputs, trace=False)[0]



# revision 3
# speedup vs baseline: 2.7220x; 2.7220x over previous
"""Trainium2 Bass kernel for nn_DenseAttention (sparse_attention, C=31, B=D=1024).

Strategy (class-parallel over 8 NeuronCores):
- Each core handles 4 classes (core 7: 3 real + 1 zero dummy).
- Single-term fp16 path (validated: end-to-end scale-rel absmax err ~6.5e-3
  vs the 2e-2 gate): xBT = K_c^T fp16 matmul of fp16 x with fp32 PSUM
  accumulate, bias added during the PSUM->SBUF fp16 copy; xBBx logits on the
  upper cross-domain half block, label-equality masking (multiply), then
  E = exp(logits - 200) shipped to host as fp32.
- The reference's softmax is a raw reshape [B,B,C] -> [C, B*B]: softmax groups
  are 31 chunks of 2^20 flat elements crossing class boundaries. Host computes
  the exact per-element group id g = (31*(i*B+j) + c) >> 20 and accumulates
  group sums in fp64 (bincount), then out = sum_c E_c / s_g.
- exp shift is the constant 200 (any per-group-constant shift cancels in the
  softmax ratio; 200 keeps everything in fp32 range and reproduces the
  reference's masked-element underflow-to-zero behaviour exactly).
- Class schedule is software-pipelined (m1 of class cl+1 issued before m2 of
  class cl, double-buffered xB) so the tensor queue never waits on the
  PSUM->SBUF copies between the two matmuls.
"""

import functools

import numpy as np

import concourse.mybir as mybir
import concourse.tile as tile
from concourse import bacc
from concourse.bass_utils import run_bass_kernel_spmd

C, B, D = 31, 1024, 1024
NCORES = 8
CPAD = 4
MHAT = 200.0
F32 = mybir.dt.float32
F16 = mybir.dt.float16
EXP = mybir.ActivationFunctionType.Exp
IDENT = mybir.ActivationFunctionType.Identity
ALU = mybir.AluOpType


@functools.lru_cache(maxsize=1)
def _build():
    nc = bacc.Bacc("TRN2", target_bir_lowering=False, debug=False,
                   num_devices=NCORES)
    xth_d = nc.dram_tensor("xth", [8, 128, 1024], F16, kind="ExternalInput")
    khi_d = nc.dram_tensor("khi", [CPAD, 8, 128, 1024], F16,
                           kind="ExternalInput")
    labi_d = nc.dram_tensor("labi", [128, CPAD * 8], F32, kind="ExternalInput")
    labj_d = nc.dram_tensor("labj", [128, CPAD * 512], F32,
                            kind="ExternalInput")
    bias_d = nc.dram_tensor("biasc", [128, CPAD * 8], F32, kind="ExternalInput")
    # E upper cross block per (class, i-tile); host mirrors the lower block
    oute_d = nc.dram_tensor("out_e", [128, CPAD * 4 * 512], F32,
                            kind="ExternalOutput")

    with tile.TileContext(nc) as tc:
        with (
            tc.tile_pool(name="persist", bufs=1) as pp,
            tc.tile_pool(name="kpool", bufs=2) as kp,
            tc.tile_pool(name="work", bufs=3) as wp,
            tc.tile_pool(name="psum", bufs=2, space="PSUM") as ps,
        ):
            xth_t = pp.tile([128, 8 * 1024], F16)
            xbh = [pp.tile([128, 8 * 1024], F16, name="xbh_a"),
                   pp.tile([128, 8 * 1024], F16, name="xbh_b")]
            labi_t = pp.tile([128, CPAD * 8], F32)
            labj_t = pp.tile([128, CPAD * 512], F32)
            bias_t = pp.tile([128, CPAD * 8], F32)
            b200_t = pp.tile([128, 1], F32)

            nc.sync.dma_start(out=bias_t[:], in_=bias_d[:])
            for dc in range(8):
                nc.sync.dma_start(out=xth_t[:, dc * 1024:(dc + 1) * 1024],
                                  in_=xth_d[dc])
            nc.sync.dma_start(out=labi_t[:], in_=labi_d[:])
            nc.sync.dma_start(out=labj_t[:], in_=labj_d[:])
            nc.vector.memset(b200_t[:], -MHAT)

            def emit_m1(cl):
                # xBT[e, i] = sum_d K[d,e] * xT[d,i] (+bias), fp16 out
                xb = xbh[cl % 2]
                for et in range(8):
                    kh_t = kp.tile([128, 1024], F16, tag="kh")
                    nc.gpsimd.dma_start(out=kh_t[:], in_=khi_d[cl, et])
                    pa = ps.tile([128, 512], F32, tag="p1")
                    pb = ps.tile([128, 512], F32, tag="p2")
                    pt = [pa, pb]
                    for dc in range(8):
                        w = kh_t[:, dc * 128:(dc + 1) * 128]
                        for ih in range(2):
                            nc.tensor.matmul(
                                out=pt[ih][:], lhsT=w,
                                rhs=xth_t[:, dc * 1024 + ih * 512:
                                          dc * 1024 + ih * 512 + 512],
                                start=(dc == 0), stop=(dc == 7))
                    bsl = bias_t[:, cl * 8 + et:cl * 8 + et + 1]
                    for ih in range(2):
                        osl = slice(et * 1024 + ih * 512,
                                    et * 1024 + ih * 512 + 512)
                        if ih == 0:
                            nc.scalar.activation(out=xb[:, osl], in_=pt[ih][:],
                                                 func=IDENT, bias=bsl, scale=1.0)
                        else:
                            nc.vector.tensor_scalar(out=xb[:, osl],
                                                    in0=pt[ih][:], scalar1=bsl,
                                                    scalar2=None, op0=ALU.add)

            def emit_m2(cl):
                # logits[i, j] on the upper cross block, mask, exp, ship out
                xb = xbh[cl % 2]
                for it in range(4):
                    q1 = ps.tile([128, 512], F32, tag="q1")
                    for ec in range(8):
                        ioff = ec * 1024 + it * 128
                        nc.tensor.matmul(
                            out=q1[:], lhsT=xb[:, ioff:ioff + 128],
                            rhs=xb[:, ec * 1024 + 512:ec * 1024 + 1024],
                            start=(ec == 0), stop=(ec == 7))
                    eqt = wp.tile([128, 512], F32, tag="eqt")
                    mt = wp.tile([128, 512], F32, tag="mt")
                    ext = wp.tile([128, 512], F32, tag="ext")
                    nc.vector.tensor_scalar(
                        out=eqt[:], in0=labj_t[:, cl * 512:cl * 512 + 512],
                        scalar1=labi_t[:, cl * 8 + it:cl * 8 + it + 1],
                        scalar2=None, op0=ALU.is_equal)
                    nc.vector.tensor_tensor(
                        out=mt[:], in0=q1[:], in1=eqt[:], op=ALU.mult)
                    nc.scalar.activation(out=ext[:], in_=mt[:], func=EXP,
                                         bias=b200_t[:], scale=1.0)
                    eoff = (cl * 4 + it) * 512
                    nc.sync.dma_start(out=oute_d[:, eoff:eoff + 512],
                                      in_=ext[:])

            emit_m1(0)
            for cl in range(CPAD):
                if cl + 1 < CPAD:
                    emit_m1(cl + 1)
                emit_m2(cl)

    nc.compile()
    return nc


def _core_classes():
    return [list(range(c * 4, min(c * 4 + 4, C))) for c in range(NCORES)]


def _prep_inputs(x, labels, kernel, bias):
    xT = np.ascontiguousarray(x.T).astype(np.float16)
    xth = np.ascontiguousarray(xT.reshape(8, 128, 1024))
    in_maps = []
    for classes in _core_classes():
        k4 = np.zeros((CPAD, D, D), np.float32)
        b4 = np.zeros((CPAD, D), np.float32)
        l4 = np.zeros((B, CPAD), np.int32)
        for cl, c in enumerate(classes):
            k4[cl] = kernel[c]
            b4[cl] = bias[c]
            l4[:, cl] = labels[:, c]
        khi = k4.astype(np.float16)
        # [cl, d, e] -> [cl, et(8), p(128), dc(8), e(128)] laid as [cl,8,128,1024]
        khi = khi.reshape(CPAD, 8, 128, 8, 128)          # cl, dc, p, et, e
        khi = np.ascontiguousarray(khi.transpose(0, 3, 2, 1, 4))  # cl,et,p,dc,e
        khi = khi.reshape(CPAD, 8, 128, 1024)
        labi = l4.reshape(8, 128, CPAD).transpose(1, 2, 0)      # p, cl, it
        labi = np.ascontiguousarray(labi.astype(np.float32)).reshape(
            128, CPAD * 8)
        labj = np.broadcast_to(
            l4[512:, :].T.astype(np.float32)[None, :, :], (128, CPAD, 512)
        ).reshape(128, CPAD * 512).copy()
        biasc = b4.reshape(CPAD, 8, 128).transpose(2, 0, 1)     # p, cl, et
        biasc = np.ascontiguousarray(biasc.astype(np.float32)).reshape(
            128, CPAD * 8)
        in_maps.append(dict(xth=xth, khi=khi, labi=labi, labj=labj,
                            biasc=biasc))
    return in_maps


def _assemble(results):
    # Full E per class (upper cross block shipped; E is symmetric)
    Efull = np.zeros((C, B, B), np.float32)
    for res, classes in zip(results, _core_classes()):
        eb = res["out_e"].reshape(128, CPAD, 4, 512)
        for cl, c in enumerate(classes):
            up = eb[:, cl].transpose(1, 0, 2).reshape(512, 512)
            Efull[c, :512, 512:] = up
            Efull[c, 512:, :512] = up.T
    # Exact flat-group softmax sums in fp64: element (i,j,c) lives at flat
    # position (i*B+j)*C + c; group = flat >> 20.
    idx = np.arange(B * B, dtype=np.int64)
    s = np.zeros(C, np.float64)
    for c in range(C):
        g = (idx * C + c) >> 20
        s += np.bincount(g, weights=Efull[c].reshape(-1).astype(np.float64),
                         minlength=C)
    out = np.zeros(B * B, np.float64)
    for c in range(C):
        g = (idx * C + c) >> 20
        out += Efull[c].reshape(-1).astype(np.float64) / s[g]
    return out.reshape(B, B).astype(np.float32)


def _run(inputs, trace=False):
    x = np.asarray(inputs["inputs"], np.float32)
    labels = np.asarray(inputs["labels"])
    kern = np.asarray(inputs["kernel"], np.float32)
    bias = np.asarray(inputs["bias"], np.float32)
    nc = _build()
    in_maps = _prep_inputs(x, labels, kern, bias)
    res = run_bass_kernel_spmd(nc, in_maps, core_ids=list(range(NCORES)),
                               trace=trace)
    out = _assemble(res.results)
    return out, res


def kernel(**inputs) -> np.ndarray:
    return _run(inputs, trace=False)[0]
